# revision 12
# baseline (speedup 1.0000x reference)
"""Trainium2 Bass kernel: GQA attention block.

Problem (hardcoded): B=2, T=1024, C=2048, N_HEADS=16, N_KV=4, H=128.
  q = rms_norm(x @ q_kernel); k = rms_norm(x @ k_kernel); v = x @ v_kernel
  q, k: RoPE;  logits = (q/sqrt(H)) @ k^T;  softmax (full, non-causal)
  out = (probs @ v) @ out_kernel

Sharding over 8 cores: core c -> (batch b = c//4, T-slice s = c%4 of 256
query rows).  Each core computes K/V for the full batch (the attention is
non-causal over all 1024 keys) and Q only for its slice; the per-core
[256, 2048] output slices are gathered on host.

On-chip layout is head-major/transposed: xT [C, T], qT/kT [head_dim, t].
The host rolls the key/value sequence so each core's 256 query positions
come first (softmax/AV are permutation-invariant in s; the RoPE tables are
rolled to match), which lets Q-projection read the first 256 columns of the
same resident xT used by K/V.  RMS-norm sums-of-squares become ones-matmul
column sums; RoPE's rotate-half is a constant permutation matmul on the PE
(DVE lanes cannot cross partitions), with sign and 1/sqrt(H) folded into
host-precomputed tables.  Softmax skips max-subtraction (logits are
rms-normed; |logit| < 7).  All matmuls run as float32r (fp22 mantissa).

Scheduling: K-projection accumulates in two T-halves so half-0's PSUM
drain (square/copy) overlaps half-1's matmuls; the rms-norm column sums
and sqrt are spread into the first Q heads.  V tiles and the q swap
matmuls are interleaved so the PE never waits on the q rms-norm chain.
"""
import os
from contextlib import ExitStack

import numpy as np

import concourse.bacc as bacc
import concourse.bass as bass
import concourse.tile as tile
from concourse import mybir
from concourse.bass_utils import run_bass_kernel_spmd

# problem constants
B, T, C = 2, 1024, 2048
N_HEADS, N_KV, H = 16, 4, 128
G = N_HEADS // N_KV      # 4 q heads per kv head
TL = T // 4              # 256 local q rows per core
P = 128                  # partitions
CT = C // P              # 16 contraction tiles
KM = (N_KV * H) // P     # 4 k m-tiles
ST = T // P              # 8 s-tiles
CB = 4                   # out-proj column blocks of 512
TH = T // 2              # 512: K-proj half width
F32 = mybir.dt.float32
F32R = mybir.dt.float32r
BF16 = mybir.dt.bfloat16
AF = mybir.ActivationFunctionType
EPS = 1e-6
MAX_TIMESCALE = 10000.0


def _r(ap):
    """float32r view (fp22-truncated matmul read) of an fp32 AP."""
    return ap.bitcast(F32R)


def _f(ap):
    """plain-fp32 view of an f32r AP (for DVE/ACT reads)."""
    return ap.bitcast(F32)


def build_nc():
    nc = bacc.Bacc(None, target_bir_lowering=False)
    t_xT = nc.dram_tensor("xT", [P, CT, T], BF16, kind="ExternalInput")
    t_qw = nc.dram_tensor("qw", [CT, P, CT, P], BF16, kind="ExternalInput")
    t_kw = nc.dram_tensor("kw", [CT, P, KM, P], BF16, kind="ExternalInput")
    t_vw = nc.dram_tensor("vw", [P, CT, N_KV * H], BF16, kind="ExternalInput")
    t_ow = nc.dram_tensor("ow", [CB, P, CT, 512], BF16, kind="ExternalInput")
    t_cq = nc.dram_tensor("cq", [P, TL], F32, kind="ExternalInput")
    t_sq = nc.dram_tensor("sq", [P, TL], F32, kind="ExternalInput")
    t_ck = nc.dram_tensor("ck", [P, T], F32, kind="ExternalInput")
    t_sk = nc.dram_tensor("sk", [P, T], F32, kind="ExternalInput")
    t_sw = nc.dram_tensor("sw", [P, P], F32R, kind="ExternalInput")
    t_out = nc.dram_tensor("out", [TL, C], F32, kind="ExternalOutput")

    with tile.TileContext(nc) as tc:
        _emit(tc, t_xT, t_qw, t_kw, t_vw, t_ow,
              t_cq, t_sq, t_ck, t_sk, t_sw, t_out)
    nc.compile()
    return nc


def _rope(nc, dst, src, sw_ps, ctab, stab, tmp, add_eng=None):
    """dst = src*ctab + sw_ps*stab.

    sw_ps is swap_halves(src) (PE permutation-matmul result in PSUM);
    the rotate_half sign lives in the stab table.
    """
    nc.vector.tensor_mul(dst, _f(src), ctab)
    nc.vector.tensor_mul(tmp, sw_ps, stab)
    (add_eng or nc.gpsimd).tensor_add(dst, _f(dst), tmp)


def _emit(tc, t_xT, t_qw, t_kw, t_vw, t_ow, t_cq, t_sq, t_ck, t_sk,
          t_sw, t_out):
    nc = tc.nc

    with ExitStack() as ctx:
        persist = ctx.enter_context(tc.tile_pool(name="persist", bufs=1))
        kT = persist.tile([P, KM, T], F32R)
        ones_b = persist.tile([P, P], BF16)
        nc.vector.memset(ones_b[:], 1.0)
        # manual pools: tiles allocated late so early phases keep headroom
        qTp = tc.alloc_tile_pool(name="qTp", bufs=1, side="right")
        vsbp = tc.alloc_tile_pool(name="vsbp", bufs=1, side="right")
        qT = None
        v_sb = None

        with ExitStack() as xctx:
            xtp = xctx.enter_context(tc.tile_pool(name="xt", bufs=1))
            xT = xtp.tile([P, CT, T], BF16)

            with ExitStack() as tctx:
                tabp = tctx.enter_context(tc.tile_pool(name="tab", bufs=1))
                sw = tabp.tile([P, P], F32R)
                eps_t = tabp.tile([P, 1], F32)
                ckr = tabp.tile([P, T], F32)
                skr = tabp.tile([P, T], F32)
                cqr = tabp.tile([P, TL], F32)
                sqr = tabp.tile([P, TL], F32)
                cq2 = tabp.tile([P, 2, TL], F32)
                sq2 = tabp.tile([P, 2, TL], F32)
                nc.vector.memset(eps_t[:], EPS)

                with ExitStack() as kctx:
                    wkp = kctx.enter_context(tc.tile_pool(name="wk", bufs=1))
                    ksqp = kctx.enter_context(
                        tc.tile_pool(name="ksq", bufs=2 * KM))
                    tmpp = kctx.enter_context(
                        tc.tile_pool(name="ktmp", bufs=1))
                    vwp = tc.alloc_tile_pool(name="vwp", bufs=1,
                                             side="right")
                    krawp = tc.alloc_tile_pool(name="krawp", bufs=1,
                                               side="right")
                    rtmpp = tc.alloc_tile_pool(name="rtmpp", bufs=2,
                                               side="right")

                    # sync ring order = need order: kw (per-ct, so the
                    # first matmul waits on 128 KB only), odd xT chunks,
                    # then tables (needed ~25 us in).
                    wkt = wkp.tile([P, CT, KM, P], BF16)
                    nc.sync.dma_start(out=wkt[:, 0, :, :], in_=t_kw[0])
                    # first x chunk in halves on the scalar ring: the
                    # first matmul needs only xT[:, 0, 0:512]
                    nc.scalar.dma_start(out=xT[:, 0, 0:TH],
                                        in_=t_xT[:, 0, 0:TH])
                    nc.scalar.dma_start(out=xT[:, 0, TH:T],
                                        in_=t_xT[:, 0, TH:T])
                    for ct in range(1, CT):
                        nc.sync.dma_start(out=wkt[:, ct, :, :],
                                          in_=t_kw[ct])
                    for ct in range(1, CT):
                        eng = nc.scalar if ct % 2 == 0 else nc.sync
                        eng.dma_start(out=xT[:, ct, :], in_=t_xT[:, ct, :])
                    nc.sync.dma_start(out=sw[:], in_=t_sw[:])
                    nc.sync.dma_start(out=ckr[:], in_=t_ck[:])
                    nc.sync.dma_start(out=skr[:], in_=t_sk[:])
                    nc.sync.dma_start(out=cqr[:], in_=t_cq[:])
                    nc.sync.dma_start(out=sqr[:], in_=t_sq[:])

                    kraw = krawp.tile([P, KM, T], F32R)
                    ksqs = [[], []]
                    with ExitStack() as pctx:
                        ppk = pctx.enter_context(
                            tc.tile_pool(name="ppk", bufs=KM, space="PSUM"))
                        # two T-halves: half-0's square/copy drain overlaps
                        # half-1's matmuls
                        pkt = [ppk.tile([P, TH], F32, tag="pk", name="pk")
                               for _ in range(KM)]
                        for h in range(2):
                            lo = h * TH
                            for ct in range(CT):
                                for mt in range(KM):
                                    nc.tensor.matmul(
                                        pkt[mt][:, :],
                                        wkt[:, ct, mt, :],
                                        xT[:, ct, lo:lo + TH],
                                        start=(ct == 0),
                                        stop=(ct == CT - 1))
                            for mt in range(KM):
                                ksq = ksqp.tile([P, TH], BF16, tag="ksq")
                                nc.scalar.square(ksq[:], pkt[mt][:])
                                nc.vector.tensor_copy(
                                    kraw[:, mt, lo:lo + TH], pkt[mt][:])
                                ksqs[h].append(ksq)

                    # rms-norm column sums + sqrt are emitted interleaved
                    # with the first Q heads (see below); lives on the
                    # right PSUM stack so it can release mid-Q-loop.
                    pks = tc.alloc_tile_pool(name="pks", bufs=1,
                                             space="PSUM", side="right")
                    ksum = pks.tile([P, T], F32)

                    def emit_ksum(h):
                        lo = h * TH
                        for mt in range(KM):
                            nc.tensor.matmul(
                                ksum[:, lo:lo + TH], ones_b[:],
                                ksqs[h][mt][:],
                                start=(mt == 0), stop=(mt == KM - 1))

                    def emit_krstd():
                        srt = tmpp.tile([P, T], F32, tag="srt")
                        nc.scalar.activation(srt[:], ksum[:], AF.Sqrt,
                                             bias=eps_t[:],
                                             scale=1.0 / (N_KV * H))
                        rstd = tmpp.tile([P, T], F32, tag="rstd")
                        nc.vector.reciprocal_approx_fast(out=rstd[:],
                                                         in_=srt[:])
                        nc.vector.tensor_mul(ckr[:], ckr[:], rstd[:])
                        nc.vector.tensor_mul(skr[:], skr[:], rstd[:])

                    # ------- Phase Q+V merged (k-rope interleaved) --------
                    with ExitStack() as qctx:
                        wqp = qctx.enter_context(
                            tc.tile_pool(name="wq", bufs=4))
                        qrawp = qctx.enter_context(
                            tc.tile_pool(name="qraw", bufs=1))
                        qsqp = qctx.enter_context(
                            tc.tile_pool(name="qsq", bufs=2))
                        qtmpp = qctx.enter_context(
                            tc.tile_pool(name="qtmp", bufs=1))
                        qrtmpp = qctx.enter_context(
                            tc.tile_pool(name="qrtmp", bufs=2))

                        qraw = qrawp.tile([P, N_HEADS, TL], F32R)
                        pswk = tc.alloc_tile_pool(name="pswk", bufs=3,
                                                  space="PSUM", side="right")
                        ppv = None
                        pv_tiles = []
                        vt_done = 0

                        def emit_ksw(mt, h):
                            lo = h * TH
                            ksw = pswk.tile([P, TH], F32, tag="ksw",
                                            name="ksw")
                            nc.tensor.matmul(
                                ksw[:], _r(sw[:]),
                                _r(kraw[:, mt, lo:lo + TH]),
                                start=True, stop=True)
                            rtmp = rtmpp.tile([P, TH], F32, tag="rtmp",
                                              name="rtmp")
                            _rope(nc, kT[:, mt, lo:lo + TH],
                                  kraw[:, mt, lo:lo + TH], ksw[:],
                                  ckr[:, lo:lo + TH], skr[:, lo:lo + TH],
                                  rtmp[:])

                        def emit_v_tile(tt):
                            pv = ppv.tile([P, N_KV * H], F32, tag="pv",
                                          name="pv")
                            for ct in range(CT):
                                nc.tensor.matmul(
                                    pv[:], xT[:, ct, tt * P:(tt + 1) * P],
                                    vw[:, ct, :],
                                    start=(ct == 0), stop=(ct == CT - 1))
                            nc.vector.tensor_copy(v_sb[:, tt, :], pv[:])

                        qsqs = []
                        with ExitStack() as pctx:
                            ppq = pctx.enter_context(
                                tc.tile_pool(name="ppq", bufs=2,
                                             space="PSUM"))
                            pqs = pctx.enter_context(
                                tc.tile_pool(name="pqs", bufs=1,
                                             space="PSUM"))
                            qsum = pqs.tile([P, TL], F32)
                            for mt in range(N_HEADS):
                                wqt = wqp.tile([P, CT, P], BF16, tag="wq")
                                nc.sync.dma_start(out=wqt[:], in_=t_qw[mt])
                                pq = ppq.tile([P, TL], F32, tag="pq")
                                for ct in range(CT):
                                    nc.tensor.matmul(
                                        pq[:], wqt[:, ct, :],
                                        xT[:, ct, 0:TL],
                                        start=(ct == 0), stop=(ct == CT - 1))
                                qsq = qsqp.tile([P, TL], BF16, tag="qsq")
                                nc.scalar.square(qsq[:], pq[:])
                                nc.vector.tensor_copy(qraw[:, mt, :], pq[:])
                                qsqs.append(qsq)
                                if mt == 0:
                                    emit_ksum(0)
                                elif mt == 1:
                                    emit_ksum(1)
                                    emit_krstd()
                                elif mt in (2, 3, 4, 5):
                                    kmt = mt - 2
                                    emit_ksw(kmt, 0)
                                    emit_ksw(kmt, 1)
                                    if mt == 5:
                                        pswk.release()
                                        pks.release()
                                        rtmpp.release()
                                if mt in (1, 3):
                                    # v weights ride the scalar ring, which
                                    # is idle once the x chunks are in
                                    if mt == 1:
                                        vw = vwp.tile(
                                            [P, CT, N_KV * H], BF16)
                                    for c4 in (mt - 1, mt):
                                        nc.scalar.dma_start(
                                            out=vw[:, 4 * c4:4 * c4 + 4, :],
                                            in_=t_vw[:, 4 * c4:4 * c4 + 4,
                                                     :])
                                if mt == 5:
                                    krawp.release()
                                    ppv = tc.alloc_tile_pool(
                                        name="ppv", bufs=2, space="PSUM",
                                        side="right")
                                    v_sb = vsbp.tile([P, ST, N_KV * H],
                                                     BF16, name="v_sb")
                                if mt >= 5 and mt % 2 == 1:
                                    emit_v_tile(vt_done)
                                    vt_done += 1
                                if mt >= 1:
                                    nc.tensor.matmul(
                                        qsum[:], ones_b[:],
                                        qsqs[mt - 1][:],
                                        start=(mt == 1),
                                        stop=(mt == N_HEADS))
                            # swap-matmuls hoisted before the q rms-norm
                            # join so the PE keeps busy while ACT/DVE
                            # drain the last heads
                            qT = qTp.tile([P, N_HEADS, TL], F32R, name="qT")
                            pswq = tc.alloc_tile_pool(name="pswq", bufs=3,
                                                      space="PSUM",
                                                      side="right")
                            qsws = {}

                            def emit_qsw(j):
                                qsw = pswq.tile([P, 2, TL], F32, tag="qsw",
                                                name="qsw")
                                for h in range(2):
                                    nc.tensor.matmul(
                                        qsw[:, h, :], _r(sw[:]),
                                        _r(qraw[:, 2 * j + h, :]),
                                        start=True, stop=True)
                                qsws[j] = qsw

                            def emit_qrope(j):
                                qtmp = qrtmpp.tile([P, 2, TL], F32,
                                                   tag="qrtmp", name="qrtmp")
                                _rope(nc, qT[:, 2 * j:2 * j + 2, :],
                                      qraw[:, 2 * j:2 * j + 2, :],
                                      qsws[j][:], cq2[:], sq2[:], qtmp[:],
                                      add_eng=nc.vector)

                            for j in range(3):
                                emit_qsw(j)
                            nc.tensor.matmul(qsum[:], ones_b[:],
                                             qsqs[N_HEADS - 1][:],
                                             start=False, stop=True)
                            srtq = qtmpp.tile([P, TL], F32, tag="srtq")
                            nc.scalar.activation(srtq[:], qsum[:], AF.Sqrt,
                                                 bias=eps_t[:],
                                                 scale=1.0 / (N_HEADS * H))
                        rstdq = qtmpp.tile([P, TL], F32, tag="rstdq")
                        nc.vector.reciprocal_approx_fast(out=rstdq[:],
                                                         in_=srtq[:])
                        for h in range(2):
                            nc.vector.tensor_mul(cq2[:, h, :], cqr[:],
                                                 rstdq[:])
                            nc.vector.tensor_mul(sq2[:, h, :], sqr[:],
                                                 rstdq[:])
                        # rope pairs stream on DVE while the PE chews the
                        # last v tiles and remaining swap matmuls
                        emit_qrope(0)
                        emit_qrope(1)
                        emit_v_tile(6)
                        emit_qsw(3)
                        emit_qsw(4)
                        emit_qrope(2)
                        emit_qrope(3)
                        emit_v_tile(7)
                        emit_qsw(5)
                        emit_qsw(6)
                        emit_qsw(7)
                        emit_qrope(4)
                        emit_qrope(5)
                        emit_qrope(6)
                        emit_qrope(7)
                        pswq.release()
                        ppv.release()
                        vwp.release()

        # ---------------- Phase A: attention ------------------------------
        with ExitStack() as actx:
            attnp = actx.enter_context(tc.tile_pool(name="attn", bufs=1))
            encT = attnp.tile([P, N_HEADS, TL], BF16)
            owp = actx.enter_context(tc.tile_pool(name="ow", bufs=CB))
            # prefetch all out-proj weight blocks during attention
            owts = []
            for cb in range(CB):
                owt = owp.tile([P, CT, 512], BF16, tag="ow", name="ow")
                nc.sync.dma_start(out=owt[:], in_=t_ow[cb])
                owts.append(owt)

            with ExitStack() as kvctx:
                expp = kvctx.enter_context(tc.tile_pool(name="exp", bufs=1))
                rcpp = kvctx.enter_context(tc.tile_pool(name="rcp", bufs=2))
                lp = kvctx.enter_context(
                    tc.tile_pool(name="lp", bufs=3, space="PSUM"))
                sp = kvctx.enter_context(
                    tc.tile_pool(name="sp", bufs=1, space="PSUM"))
                ep = kvctx.enter_context(
                    tc.tile_pool(name="ep", bufs=1, space="PSUM"))

                for kh in range(N_KV):
                    ex = expp.tile([P, ST, G, TL], BF16, tag="ex")
                    for pair in range(2):
                        hlo = 2 * pair
                        q_rhs = qT[:, G * kh + hlo:G * kh + hlo + 2, :]
                        for st2 in range(ST // 2):
                            L = lp.tile([P, 2, 2, TL], F32, tag="L")
                            for j in range(2):
                                st = st2 * 2 + j
                                nc.tensor.matmul(
                                    L[:, j, :, :],
                                    _r(kT[:, kh, st * P:(st + 1) * P]),
                                    _r(q_rhs), start=True, stop=True)
                            nc.scalar.activation(
                                ex[:, st2 * 2:st2 * 2 + 2, hlo:hlo + 2, :],
                                L[:], AF.Exp)
                        # softmax denominators, replicated over partitions
                        S = sp.tile([P, 2, TL], F32, tag="S")
                        for st in range(ST):
                            nc.tensor.matmul(
                                S[:], ones_b[:],
                                ex[:, st, hlo:hlo + 2, :],
                                start=(st == 0), stop=(st == ST - 1))
                        # probs @ v  (unnormalized)
                        E = ep.tile([P, 2, TL], F32, tag="E")
                        for st in range(ST):
                            nc.tensor.matmul(
                                E[:], v_sb[:, st, kh * H:(kh + 1) * H],
                                ex[:, st, hlo:hlo + 2, :],
                                start=(st == 0), stop=(st == ST - 1))
                        # normalize while draining PSUM -> SBUF
                        rcp = rcpp.tile([P, 2, TL], F32, tag="rcp")
                        nc.vector.reciprocal_approx_fast(out=rcp[:],
                                                         in_=S[:])
                        nc.vector.tensor_mul(
                            encT[:, G * kh + hlo:G * kh + hlo + 2, :],
                            E[:], rcp[:])

            # ---------------- Phase O: output projection ------------------
            with ExitStack() as octx:
                otp = octx.enter_context(tc.tile_pool(name="ot", bufs=6))
                pop = octx.enter_context(
                    tc.tile_pool(name="po", bufs=4, space="PSUM"))
                for cb in range(CB):
                    owt = owts[cb]
                    for tt in range(TL // P):
                        PO = pop.tile([P, 512], F32, tag="PO")
                        for mt in range(CT):
                            nc.tensor.matmul(
                                PO[:], encT[:, mt, tt * P:(tt + 1) * P],
                                owt[:, mt, :],
                                start=(mt == 0), stop=(mt == CT - 1))
                        o = otp.tile([P, 512], F32, tag="o")
                        nc.vector.tensor_copy(o[:], PO[:])
                        eng = nc.sync if (cb * 2 + tt) % 2 == 0 else \
                            nc.scalar
                        eng.dma_start(
                            out=t_out[tt * P:(tt + 1) * P,
                                      cb * 512:(cb + 1) * 512],
                            in_=o[:])
        vsbp.release()
        qTp.release()


# ---------------------------------------------------------------------------
# host side: input prep, sharding, gather
# ---------------------------------------------------------------------------

def _tables():
    fraction = np.arange(0, H, 2, dtype=np.float32) / np.float32(H)
    inv_freq = (1.0 / (MAX_TIMESCALE ** fraction)).astype(np.float32)
    sinusoid = np.arange(T, dtype=np.float32)[:, None] * inv_freq[None, :]
    sinusoid = np.concatenate([sinusoid, sinusoid], axis=-1)  # [T, H]
    sinT = np.sin(sinusoid).T.astype(np.float32)              # [H, T]
    cosT = np.cos(sinusoid).T.astype(np.float32)
    sin_signed = np.concatenate([-sinT[:H // 2], sinT[H // 2:]], axis=0)
    scale = np.float32(1.0) / np.sqrt(np.float32(H)).astype(np.float32)
    return (cosT.copy(), sin_signed.copy(),
            (cosT * scale).astype(np.float32),
            (sin_signed * scale).astype(np.float32))


def make_in_maps(x, q_kernel, k_kernel, v_kernel, out_kernel):
    x = np.ascontiguousarray(np.asarray(x, dtype=np.float32))
    qk = np.asarray(q_kernel, dtype=np.float32)
    kk = np.asarray(k_kernel, dtype=np.float32)
    vk = np.asarray(v_kernel, dtype=np.float32)
    ok = np.asarray(out_kernel, dtype=np.float32)

    import ml_dtypes
    bf16 = ml_dtypes.bfloat16
    qw = np.ascontiguousarray(
        qk.reshape(CT, P, CT, P).transpose(2, 1, 0, 3).astype(bf16))
    kw = np.ascontiguousarray(kk.reshape(CT, P, KM, P).astype(bf16))
    vw = np.ascontiguousarray(
        vk.reshape(CT, P, N_KV * H).transpose(1, 0, 2).astype(bf16))
    ow = np.ascontiguousarray(
        ok.reshape(CT, P, CB, 512).transpose(2, 1, 0, 3).astype(
            np.dtype(bf16)))
    ck_h, sk_h, cq_full, sq_full = _tables()
    sw_h = np.zeros((P, P), np.float32)
    sw_h[(np.arange(P) + P // 2) % P, np.arange(P)] = 1.0

    xt = [np.ascontiguousarray(
        x[b].T.reshape(CT, P, T).transpose(1, 0, 2)) for b in range(B)]

    in_maps = []
    for core in range(8):
        b, s = divmod(core, 4)
        t0 = s * TL
        # roll the key/value sequence so this core's query block is first;
        # softmax over s is permutation-invariant, RoPE tables roll along.
        in_maps.append({
            "xT": np.ascontiguousarray(
                np.roll(xt[b], -t0, axis=2).astype(bf16)),
            "qw": qw, "kw": kw, "vw": vw, "ow": ow,
            "cq": np.ascontiguousarray(
                np.roll(cq_full, -t0, axis=1)[:, :TL]),
            "sq": np.ascontiguousarray(
                np.roll(sq_full, -t0, axis=1)[:, :TL]),
            "ck": np.ascontiguousarray(np.roll(ck_h, -t0, axis=1)),
            "sk": np.ascontiguousarray(np.roll(sk_h, -t0, axis=1)),
            "sw": sw_h,
        })
    return in_maps


def _install_trace_shim():
    """Dev-only (KERNEL_TRACE=1): register the NTFF profile hook that this
    agent image's antenv lacks, and skip the artifact cloud upload."""
    import sys
    import types
    try:
        from antenv import axon_hooks  # noqa: F401
        ok = True
    except ImportError:
        try:
            from trn_agent_boot.trn_boot import _ntff_profile_via_ctypes
            hook = _ntff_profile_via_ctypes("/opt/axon/libaxon_pjrt.so")
            m = types.ModuleType("antenv.axon_hooks")
            m.get_axon_ntff_profile_hook = lambda: hook
            m.set_axon_ntff_profile_hook = lambda h: None
            sys.modules["antenv.axon_hooks"] = m
            ok = True
        except Exception as e:  # profiling unavailable; still run
            print(f"trace shim failed: {e!r}")
            ok = False
    if ok:
        import concourse.bass_utils as bu
        bu.upload_artifacts = lambda tmpdir: tmpdir
    return ok


def kernel(x, q_kernel, k_kernel, v_kernel, out_kernel):
    in_maps = make_in_maps(x, q_kernel, k_kernel, v_kernel, out_kernel)
    nc = build_nc()
    trace = bool(os.environ.get("KERNEL_TRACE"))
    kwargs = {}
    if trace:
        trace = _install_trace_shim()
        if trace:
            tdir = os.environ.get("KERNEL_TRACE_DIR")
            if tdir:
                os.makedirs(tdir, exist_ok=True)
                kwargs["tmpdir"] = tdir
    res = run_bass_kernel_spmd(nc, in_maps, core_ids=list(range(8)),
                               trace=trace, **kwargs)
    out = np.zeros((B, T, C), np.float32)
    for core in range(8):
        b, s = divmod(core, 4)
        out[b, s * TL:(s + 1) * TL] = res.results[core]["out"]
    if trace:
        kernel.last_exec_time_ns = res.exec_time_ns
        kernel.last_profile = res.profile_json
    return out


# revision 24
# speedup vs baseline: 1.0947x; 1.0947x over previous
"""Trainium2 Bass kernel: GQA attention block.

Problem (hardcoded): B=2, T=1024, C=2048, N_HEADS=16, N_KV=4, H=128.
  q = rms_norm(x @ q_kernel); k = rms_norm(x @ k_kernel); v = x @ v_kernel
  q, k: RoPE;  logits = (q/sqrt(H)) @ k^T;  softmax (full, non-causal)
  out = (probs @ v) @ out_kernel

Sharding over 8 cores: core c -> (batch b = c//4, T-slice s = c%4 of 256
query rows).  Each core computes K/V for the full batch (the attention is
non-causal over all 1024 keys) and Q only for its slice; the per-core
[256, 2048] output slices are gathered on host.

On-chip layout is head-major/transposed: xT [C, T], qT/kT [head_dim, t].
The host rolls the key/value sequence so each core's 256 query positions
come first (softmax/AV are permutation-invariant in s; the RoPE tables are
rolled to match), which lets Q-projection read the first 256 columns of the
same resident xT used by K/V.  RMS-norm sums-of-squares become ones-matmul
column sums; RoPE's rotate-half is a constant permutation matmul on the PE
(DVE lanes cannot cross partitions), with sign and 1/sqrt(H) folded into
host-precomputed tables.  Softmax skips max-subtraction (logits are
rms-normed; |logit| < 7).

Scheduling: K-projection accumulates in two T-halves so half-0's PSUM
drain overlaps half-1's matmuls.  Q RoPE is applied per head-pair with
UNSCALED tables as soon as each projection lands (the rms-norm rstd is a
per-token scalar that commutes through the rotation); the rstd is folded
in afterwards by one DVE mul per pair, so nothing but those muls sits on
the q-norm critical path.  qT/kT are bf16 (cheap LDWEIGHTS for the logit
matmuls).  Softmax denominators pre-reduce pairs of s-tiles on DVE/GpSimd
before the ones-matmul.
"""
import os
from contextlib import ExitStack

import numpy as np

import concourse.bacc as bacc
import concourse.bass as bass
import concourse.tile as tile
from concourse import mybir
from concourse.bass_utils import run_bass_kernel_spmd

# problem constants
B, T, C = 2, 1024, 2048
N_HEADS, N_KV, H = 16, 4, 128
G = N_HEADS // N_KV      # 4 q heads per kv head
TL = T // 4              # 256 local q rows per core
P = 128                  # partitions
CT = C // P              # 16 contraction tiles
KM = (N_KV * H) // P     # 4 k m-tiles
ST = T // P              # 8 s-tiles
CB = 4                   # out-proj column blocks of 512
TH = T // 2              # 512: K-proj half width
F32 = mybir.dt.float32
F32R = mybir.dt.float32r
BF16 = mybir.dt.bfloat16
AF = mybir.ActivationFunctionType
EPS = 1e-6
MAX_TIMESCALE = 10000.0


def _r(ap):
    """float32r view (fp22-truncated matmul read) of an fp32 AP."""
    return ap.bitcast(F32R)


def _f(ap):
    """plain-fp32 view of an f32r AP (for DVE/ACT reads)."""
    return ap.bitcast(F32)


def build_nc():
    nc = bacc.Bacc(None, target_bir_lowering=False)
    t_xT = nc.dram_tensor("xT", [P, CT, T], BF16, kind="ExternalInput")
    t_qw = nc.dram_tensor("qw", [CT, P, CT, P], BF16, kind="ExternalInput")
    t_kw = nc.dram_tensor("kw", [CT, P, KM, P], BF16, kind="ExternalInput")
    t_vw = nc.dram_tensor("vw", [P, CT, N_KV * H], BF16, kind="ExternalInput")
    t_ow = nc.dram_tensor("ow", [CB, P, CT, 512], BF16, kind="ExternalInput")
    t_cq = nc.dram_tensor("cq", [P, TL], F32, kind="ExternalInput")
    t_sq = nc.dram_tensor("sq", [P, TL], F32, kind="ExternalInput")
    t_ck = nc.dram_tensor("ck", [P, T], F32, kind="ExternalInput")
    t_sk = nc.dram_tensor("sk", [P, T], F32, kind="ExternalInput")
    t_sw = nc.dram_tensor("sw", [P, P], F32R, kind="ExternalInput")
    t_out = nc.dram_tensor("out", [TL, C], F32, kind="ExternalOutput")

    with tile.TileContext(nc) as tc:
        _emit(tc, t_xT, t_qw, t_kw, t_vw, t_ow,
              t_cq, t_sq, t_ck, t_sk, t_sw, t_out)
    nc.compile()
    return nc


def _emit(tc, t_xT, t_qw, t_kw, t_vw, t_ow, t_cq, t_sq, t_ck, t_sk,
          t_sw, t_out):
    nc = tc.nc

    with ExitStack() as ctx:
        persist = ctx.enter_context(tc.tile_pool(name="persist", bufs=1))
        kT = persist.tile([P, KM, T], BF16)
        ones_b = persist.tile([P, P], BF16)
        nc.vector.memset(ones_b[:], 1.0)
        # manual pools: tiles allocated late so early phases keep headroom
        qTp = tc.alloc_tile_pool(name="qTp", bufs=1, side="right")
        vsbp = tc.alloc_tile_pool(name="vsbp", bufs=1, side="right")
        qT = None
        v_sb = None

        with ExitStack() as xctx:
            xtp = xctx.enter_context(tc.tile_pool(name="xt", bufs=1))
            xT = xtp.tile([P, CT, T], BF16)

            with ExitStack() as tctx:
                tabp = tctx.enter_context(tc.tile_pool(name="tab", bufs=1))
                sw = tabp.tile([P, P], F32R)
                eps_t = tabp.tile([P, 1], F32)
                ckr = tabp.tile([P, T], F32)
                skr = tabp.tile([P, T], F32)
                cq2 = tabp.tile([P, 2, TL], F32)
                sq2 = tabp.tile([P, 2, TL], F32)
                srt = tabp.tile([P, T], F32)
                nc.vector.memset(eps_t[:], EPS)

                with ExitStack() as kctx:
                    wkp = kctx.enter_context(tc.tile_pool(name="wk", bufs=1))
                    ksqp = kctx.enter_context(
                        tc.tile_pool(name="ksq", bufs=2 * KM))
                    tmpp = kctx.enter_context(
                        tc.tile_pool(name="ktmp", bufs=1))
                    vwp = tc.alloc_tile_pool(name="vwp", bufs=1,
                                             side="right")
                    krawp = tc.alloc_tile_pool(name="krawp", bufs=1,
                                               side="right")
                    rtmpp = tc.alloc_tile_pool(name="rtmpp", bufs=4,
                                               side="right")

                    # sync ring: kw ct-chunks interleaved with the odd xT
                    # chunks (so neither queues behind the other); tables
                    # afterwards (needed ~25us in).  scalar ring: even xT.
                    wkt = wkp.tile([P, CT, KM, P], BF16)
                    nc.sync.dma_start(out=wkt[:, 0, :, :], in_=t_kw[0])
                    nc.scalar.dma_start(out=xT[:, 0, 0:TH],
                                        in_=t_xT[:, 0, 0:TH])
                    nc.scalar.dma_start(out=xT[:, 0, TH:T],
                                        in_=t_xT[:, 0, TH:T])
                    for ct in range(1, CT):
                        nc.sync.dma_start(out=wkt[:, ct, :, :],
                                          in_=t_kw[ct])
                        if ct % 2 == 1:
                            nc.sync.dma_start(out=xT[:, ct, :],
                                              in_=t_xT[:, ct, :])
                        else:
                            nc.scalar.dma_start(out=xT[:, ct, :],
                                                in_=t_xT[:, ct, :])
                    nc.sync.dma_start(out=sw[:], in_=t_sw[:])
                    nc.sync.dma_start(out=ckr[:], in_=t_ck[:])
                    nc.sync.dma_start(out=skr[:], in_=t_sk[:])
                    for h in range(2):
                        nc.sync.dma_start(out=cq2[:, h, :], in_=t_cq[:])
                        nc.sync.dma_start(out=sq2[:, h, :], in_=t_sq[:])

                    kraw = krawp.tile([P, KM, T], F32R)
                    ksqs = [[], []]
                    with ExitStack() as pctx:
                        ppk = pctx.enter_context(
                            tc.tile_pool(name="ppk", bufs=KM, space="PSUM"))
                        # two T-halves: half-0's square/copy drain overlaps
                        # half-1's matmuls
                        pkt = [ppk.tile([P, TH], F32, tag="pk", name="pk")
                               for _ in range(KM)]
                        for h in range(2):
                            lo = h * TH
                            for ct in range(CT):
                                for mt in range(KM):
                                    nc.tensor.matmul(
                                        pkt[mt][:, :],
                                        wkt[:, ct, mt, :],
                                        xT[:, ct, lo:lo + TH],
                                        start=(ct == 0),
                                        stop=(ct == CT - 1))
                            for mt in range(KM):
                                ksq = ksqp.tile([P, TH], BF16, tag="ksq")
                                nc.scalar.square(ksq[:], pkt[mt][:])
                                nc.vector.tensor_copy(
                                    kraw[:, mt, lo:lo + TH], pkt[mt][:])
                                ksqs[h].append(ksq)

                    # ------- Phase Q+V merged (k-rope interleaved) --------
                    with ExitStack() as qctx:
                        wqp = qctx.enter_context(
                            tc.tile_pool(name="wq", bufs=4))
                        qrawp = qctx.enter_context(
                            tc.tile_pool(name="qraw", bufs=1))
                        qtup = qctx.enter_context(
                            tc.tile_pool(name="qtu", bufs=1))
                        qsqp = qctx.enter_context(
                            tc.tile_pool(name="qsq", bufs=2))
                        qtmpp = qctx.enter_context(
                            tc.tile_pool(name="qtmp", bufs=1))
                        qrtmpp = qctx.enter_context(
                            tc.tile_pool(name="qrtmp", bufs=2))

                        qraw = qrawp.tile([P, N_HEADS, TL], F32R)
                        qTu = qtup.tile([P, N_HEADS, TL], F32)
                        # PSUM pools for the swap matmuls and v tiles are
                        # created at mt==2 so their (statically reserved)
                        # banks don't overlap the k-norm column sums.
                        pks = pswk = pswq = ppv = None
                        vt_done = 0

                        def emit_ksum(h):
                            lo = h * TH
                            ks = pks.tile([P, TH], F32, tag="ksum",
                                          name="ksum")
                            for mt in range(KM):
                                nc.tensor.matmul(
                                    ks[:], ones_b[:], ksqs[h][mt][:],
                                    start=(mt == 0), stop=(mt == KM - 1))
                            nc.scalar.activation(srt[:, lo:lo + TH], ks[:],
                                                 AF.Sqrt, bias=eps_t[:],
                                                 scale=1.0 / (N_KV * H))

                        def emit_krstd():
                            rstd = tmpp.tile([P, T], F32, tag="rstd")
                            nc.vector.reciprocal_approx_fast(out=rstd[:],
                                                             in_=srt[:])
                            nc.vector.tensor_mul(ckr[:], ckr[:], rstd[:])
                            nc.vector.tensor_mul(skr[:], skr[:], rstd[:])

                        def emit_ksw(mt, h):
                            lo = h * TH
                            ksw = pswk.tile([P, TH], F32, tag="ksw",
                                            name="ksw")
                            nc.tensor.matmul(
                                ksw[:], _r(sw[:]),
                                _r(kraw[:, mt, lo:lo + TH]),
                                start=True, stop=True)
                            # rope: kT = kraw*ck + swap(kraw)*sk  (both
                            # products in fp32 tmps; the add casts to bf16)
                            t1 = rtmpp.tile([P, TH], F32, tag="rtmp",
                                            name="rtmp")
                            t2 = rtmpp.tile([P, TH], F32, tag="rtmp",
                                            name="rtmp")
                            nc.vector.tensor_mul(t1[:], ksw[:],
                                                 skr[:, lo:lo + TH])
                            nc.vector.tensor_mul(
                                t2[:], _f(kraw[:, mt, lo:lo + TH]),
                                ckr[:, lo:lo + TH])
                            nc.gpsimd.tensor_add(kT[:, mt, lo:lo + TH],
                                                 t2[:], t1[:])

                        def emit_v_mm(tt):
                            pv = ppv.tile([P, N_KV * H], F32, tag="pv",
                                          name="pv")
                            for ct in range(CT):
                                nc.tensor.matmul(
                                    pv[:], xT[:, ct, tt * P:(tt + 1) * P],
                                    vw[:, ct, :],
                                    start=(ct == 0), stop=(ct == CT - 1))
                            return pv

                        def emit_v_tile(tt):
                            pv = emit_v_mm(tt)
                            nc.vector.tensor_copy(v_sb[:, tt, :], pv[:])

                        def emit_qsw_rope(j):
                            # swap matmul + UNSCALED rope for heads 2j,2j+1
                            qsw = pswq.tile([P, 2, TL], F32, tag="qsw",
                                            name="qsw")
                            for hh in range(2):
                                nc.tensor.matmul(
                                    qsw[:, hh, :], _r(sw[:]),
                                    _r(qraw[:, 2 * j + hh, :]),
                                    start=True, stop=True)
                            qtmp = qrtmpp.tile([P, 2, TL], F32,
                                               tag="qrtmp", name="qrtmp")
                            dst = qTu[:, 2 * j:2 * j + 2, :]
                            nc.vector.tensor_mul(qtmp[:], qsw[:], sq2[:])
                            nc.vector.tensor_mul(
                                dst, _f(qraw[:, 2 * j:2 * j + 2, :]),
                                cq2[:])
                            nc.gpsimd.tensor_add(dst, dst, qtmp[:])

                        qsqs = []
                        with ExitStack() as pctx:
                            ppq = pctx.enter_context(
                                tc.tile_pool(name="ppq", bufs=2,
                                             space="PSUM"))
                            pqs = pctx.enter_context(
                                tc.tile_pool(name="pqs", bufs=1,
                                             space="PSUM"))
                            qsum = pqs.tile([P, TL], F32)
                            pks = tc.alloc_tile_pool(name="pks", bufs=2,
                                                     space="PSUM")
                            for mt in range(N_HEADS):
                                wqt = wqp.tile([P, CT, P], BF16, tag="wq")
                                nc.sync.dma_start(out=wqt[:], in_=t_qw[mt])
                                pq = ppq.tile([P, TL], F32, tag="pq")
                                for ct in range(CT):
                                    nc.tensor.matmul(
                                        pq[:], wqt[:, ct, :],
                                        xT[:, ct, 0:TL],
                                        start=(ct == 0), stop=(ct == CT - 1))
                                qsq = qsqp.tile([P, TL], BF16, tag="qsq")
                                nc.scalar.square(qsq[:], pq[:])
                                nc.vector.tensor_copy(qraw[:, mt, :], pq[:])
                                qsqs.append(qsq)
                                if mt == 0:
                                    emit_ksum(0)
                                elif mt == 1:
                                    emit_ksum(1)
                                    emit_krstd()
                                    pks.release()
                                elif mt == 2:
                                    pswk = tc.alloc_tile_pool(
                                        name="pswk", bufs=2, space="PSUM",
                                        side="right")
                                    pswq = tc.alloc_tile_pool(
                                        name="pswq", bufs=1, space="PSUM",
                                        side="right")
                                    ppv = tc.alloc_tile_pool(
                                        name="ppv", bufs=2, space="PSUM",
                                        side="right")
                                if mt in (2, 4, 6, 8):
                                    kmt = (mt - 2) // 2
                                    emit_ksw(kmt, 0)
                                    emit_ksw(kmt, 1)
                                if mt in (1, 3):
                                    # v weights ride the scalar ring, idle
                                    # once the x chunks are in
                                    if mt == 1:
                                        vw = vwp.tile(
                                            [P, CT, N_KV * H], BF16)
                                    for c4 in (mt - 1, mt):
                                        nc.scalar.dma_start(
                                            out=vw[:, 4 * c4:4 * c4 + 4, :],
                                            in_=t_vw[:, 4 * c4:4 * c4 + 4,
                                                     :])
                                if mt >= 2 and mt % 2 == 0:
                                    emit_qsw_rope((mt - 2) // 2)
                                if mt == 5:
                                    v_sb = vsbp.tile([P, ST, N_KV * H],
                                                     BF16, name="v_sb")
                                if mt >= 5 and mt % 2 == 1:
                                    emit_v_tile(vt_done)
                                    vt_done += 1
                                if mt >= 1:
                                    nc.tensor.matmul(
                                        qsum[:], ones_b[:],
                                        qsqs[mt - 1][:],
                                        start=(mt == 1),
                                        stop=(mt == N_HEADS))
                            nc.tensor.matmul(qsum[:], ones_b[:],
                                             qsqs[N_HEADS - 1][:],
                                             start=False, stop=True)
                            srtq = qtmpp.tile([P, TL], F32, tag="srtq")
                            nc.scalar.activation(srtq[:], qsum[:], AF.Sqrt,
                                                 bias=eps_t[:],
                                                 scale=1.0 / (N_HEADS * H))
                        # tail: last swap+rope, last v tiles as PE filler,
                        # then fold rstd_q into qT (one DVE mul per pair).
                        # v copies are deferred so the DVE reaches the
                        # rstd-fold muls (which gate attention) first.
                        qT = qTp.tile([P, N_HEADS, TL], BF16, name="qT")
                        emit_qsw_rope(7)
                        pv6 = emit_v_mm(6)
                        rstdq2 = qtmpp.tile([P, 2, TL], F32, tag="rstdq2")
                        for h in range(2):
                            nc.vector.reciprocal_approx_fast(
                                out=rstdq2[:, h, :], in_=srtq[:])
                        for j in range(N_HEADS // 2):
                            nc.vector.tensor_mul(
                                qT[:, 2 * j:2 * j + 2, :],
                                qTu[:, 2 * j:2 * j + 2, :], rstdq2[:])
                            if j == 1:
                                nc.vector.tensor_copy(v_sb[:, 6, :],
                                                      pv6[:])
                                pv7 = emit_v_mm(7)
                            elif j == 5:
                                nc.vector.tensor_copy(v_sb[:, 7, :],
                                                      pv7[:])
                        ppv.release()
                        pswq.release()
                        pswk.release()
                        rtmpp.release()
                        krawp.release()
                        vwp.release()

        # ---------------- Phase A: attention ------------------------------
        with ExitStack() as actx:
            attnp = actx.enter_context(tc.tile_pool(name="attn", bufs=1))
            encT = attnp.tile([P, N_HEADS, TL], BF16)
            owp = actx.enter_context(tc.tile_pool(name="ow", bufs=CB))
            # prefetch all out-proj weight blocks during attention
            owts = []
            for cb in range(CB):
                owt = owp.tile([P, CT, 512], BF16, tag="ow", name="ow")
                nc.sync.dma_start(out=owt[:], in_=t_ow[cb])
                owts.append(owt)

            with ExitStack() as kvctx:
                expp = kvctx.enter_context(tc.tile_pool(name="exp", bufs=1))
                exsp = kvctx.enter_context(tc.tile_pool(name="exs", bufs=2))
                rcpp = kvctx.enter_context(tc.tile_pool(name="rcp", bufs=2))
                lp = kvctx.enter_context(
                    tc.tile_pool(name="lp", bufs=3, space="PSUM"))
                sp = kvctx.enter_context(
                    tc.tile_pool(name="sp", bufs=1, space="PSUM"))
                ep = kvctx.enter_context(
                    tc.tile_pool(name="ep", bufs=1, space="PSUM"))

                for kh in range(N_KV):
                    ex = expp.tile([P, ST, G, TL], BF16, tag="ex")
                    for pair in range(2):
                        hlo = 2 * pair
                        q_rhs = qT[:, G * kh + hlo:G * kh + hlo + 2, :]
                        for st2 in range(ST // 2):
                            L = lp.tile([P, 2, 2, TL], F32, tag="L")
                            for j in range(2):
                                st = st2 * 2 + j
                                nc.tensor.matmul(
                                    L[:, j, :, :],
                                    kT[:, kh, st * P:(st + 1) * P],
                                    q_rhs, start=True, stop=True)
                            nc.scalar.activation(
                                ex[:, st2 * 2:st2 * 2 + 2, hlo:hlo + 2, :],
                                L[:], AF.Exp)
                        # pre-reduce s-tile pairs off the PE, then ones-
                        # matmul the 4 partials into the softmax denominator
                        exs = exsp.tile([P, 4, 2, TL], BF16, tag="exs")
                        for i in range(4):
                            eng = nc.vector if i % 2 == 0 else nc.gpsimd
                            eng.tensor_add(
                                exs[:, i, :, :],
                                ex[:, 2 * i, hlo:hlo + 2, :],
                                ex[:, 2 * i + 1, hlo:hlo + 2, :])
                        S = sp.tile([P, 2, TL], F32, tag="S")
                        for i in range(4):
                            nc.tensor.matmul(
                                S[:], ones_b[:], exs[:, i, :, :],
                                start=(i == 0), stop=(i == 3))
                        # probs @ v  (unnormalized)
                        E = ep.tile([P, 2, TL], F32, tag="E")
                        for st in range(ST):
                            nc.tensor.matmul(
                                E[:], v_sb[:, st, kh * H:(kh + 1) * H],
                                ex[:, st, hlo:hlo + 2, :],
                                start=(st == 0), stop=(st == ST - 1))
                        # normalize while draining PSUM -> SBUF
                        rcp = rcpp.tile([P, 2, TL], F32, tag="rcp")
                        nc.vector.reciprocal_approx_fast(out=rcp[:],
                                                         in_=S[:])
                        nc.vector.tensor_mul(
                            encT[:, G * kh + hlo:G * kh + hlo + 2, :],
                            E[:], rcp[:])

            # ---------------- Phase O: output projection ------------------
            with ExitStack() as octx:
                otp = octx.enter_context(tc.tile_pool(name="ot", bufs=6))
                pop = octx.enter_context(
                    tc.tile_pool(name="po", bufs=4, space="PSUM"))
                for cb in range(CB):
                    owt = owts[cb]
                    for tt in range(TL // P):
                        PO = pop.tile([P, 512], F32, tag="PO")
                        for mt in range(CT):
                            nc.tensor.matmul(
                                PO[:], encT[:, mt, tt * P:(tt + 1) * P],
                                owt[:, mt, :],
                                start=(mt == 0), stop=(mt == CT - 1))
                        o = otp.tile([P, 512], F32, tag="o")
                        nc.vector.tensor_copy(o[:], PO[:])
                        eng = nc.sync if (cb * 2 + tt) % 2 == 0 else \
                            nc.scalar
                        eng.dma_start(
                            out=t_out[tt * P:(tt + 1) * P,
                                      cb * 512:(cb + 1) * 512],
                            in_=o[:])
        vsbp.release()
        qTp.release()


# ---------------------------------------------------------------------------
# host side: input prep, sharding, gather
# ---------------------------------------------------------------------------

def _tables():
    fraction = np.arange(0, H, 2, dtype=np.float32) / np.float32(H)
    inv_freq = (1.0 / (MAX_TIMESCALE ** fraction)).astype(np.float32)
    sinusoid = np.arange(T, dtype=np.float32)[:, None] * inv_freq[None, :]
    sinusoid = np.concatenate([sinusoid, sinusoid], axis=-1)  # [T, H]
    sinT = np.sin(sinusoid).T.astype(np.float32)              # [H, T]
    cosT = np.cos(sinusoid).T.astype(np.float32)
    sin_signed = np.concatenate([-sinT[:H // 2], sinT[H // 2:]], axis=0)
    scale = np.float32(1.0) / np.sqrt(np.float32(H)).astype(np.float32)
    return (cosT.copy(), sin_signed.copy(),
            (cosT * scale).astype(np.float32),
            (sin_signed * scale).astype(np.float32))


def make_in_maps(x, q_kernel, k_kernel, v_kernel, out_kernel):
    x = np.ascontiguousarray(np.asarray(x, dtype=np.float32))
    qk = np.asarray(q_kernel, dtype=np.float32)
    kk = np.asarray(k_kernel, dtype=np.float32)
    vk = np.asarray(v_kernel, dtype=np.float32)
    ok = np.asarray(out_kernel, dtype=np.float32)

    import ml_dtypes
    bf16 = ml_dtypes.bfloat16
    qw = np.ascontiguousarray(
        qk.reshape(CT, P, CT, P).transpose(2, 1, 0, 3).astype(bf16))
    kw = np.ascontiguousarray(kk.reshape(CT, P, KM, P).astype(bf16))
    vw = np.ascontiguousarray(
        vk.reshape(CT, P, N_KV * H).transpose(1, 0, 2).astype(bf16))
    ow = np.ascontiguousarray(
        ok.reshape(CT, P, CB, 512).transpose(2, 1, 0, 3).astype(
            np.dtype(bf16)))
    ck_h, sk_h, cq_full, sq_full = _tables()
    sw_h = np.zeros((P, P), np.float32)
    sw_h[(np.arange(P) + P // 2) % P, np.arange(P)] = 1.0

    xt = [np.ascontiguousarray(
        x[b].T.reshape(CT, P, T).transpose(1, 0, 2)) for b in range(B)]

    in_maps = []
    for core in range(8):
        b, s = divmod(core, 4)
        t0 = s * TL
        # roll the key/value sequence so this core's query block is first;
        # softmax over s is permutation-invariant, RoPE tables roll along.
        in_maps.append({
            "xT": np.ascontiguousarray(
                np.roll(xt[b], -t0, axis=2).astype(bf16)),
            "qw": qw, "kw": kw, "vw": vw, "ow": ow,
            "cq": np.ascontiguousarray(
                np.roll(cq_full, -t0, axis=1)[:, :TL]),
            "sq": np.ascontiguousarray(
                np.roll(sq_full, -t0, axis=1)[:, :TL]),
            "ck": np.ascontiguousarray(np.roll(ck_h, -t0, axis=1)),
            "sk": np.ascontiguousarray(np.roll(sk_h, -t0, axis=1)),
            "sw": sw_h,
        })
    return in_maps


def _install_trace_shim():
    """Dev-only (KERNEL_TRACE=1): register the NTFF profile hook that this
    agent image's antenv lacks, and skip the artifact cloud upload."""
    import sys
    import types
    try:
        from antenv import axon_hooks  # noqa: F401
        ok = True
    except ImportError:
        try:
            from trn_agent_boot.trn_boot import _ntff_profile_via_ctypes
            hook = _ntff_profile_via_ctypes("/opt/axon/libaxon_pjrt.so")
            m = types.ModuleType("antenv.axon_hooks")
            m.get_axon_ntff_profile_hook = lambda: hook
            m.set_axon_ntff_profile_hook = lambda h: None
            sys.modules["antenv.axon_hooks"] = m
            ok = True
        except Exception as e:  # profiling unavailable; still run
            print(f"trace shim failed: {e!r}")
            ok = False
    if ok:
        import concourse.bass_utils as bu
        bu.upload_artifacts = lambda tmpdir: tmpdir
    return ok


def kernel(x, q_kernel, k_kernel, v_kernel, out_kernel):
    in_maps = make_in_maps(x, q_kernel, k_kernel, v_kernel, out_kernel)
    nc = build_nc()
    trace = bool(os.environ.get("KERNEL_TRACE"))
    kwargs = {}
    if trace:
        trace = _install_trace_shim()
        if trace:
            tdir = os.environ.get("KERNEL_TRACE_DIR")
            if tdir:
                os.makedirs(tdir, exist_ok=True)
                kwargs["tmpdir"] = tdir
    res = run_bass_kernel_spmd(nc, in_maps, core_ids=list(range(8)),
                               trace=trace, **kwargs)
    out = np.zeros((B, T, C), np.float32)
    for core in range(8):
        b, s = divmod(core, 4)
        out[b, s * TL:(s + 1) * TL] = res.results[core]["out"]
    if trace:
        kernel.last_exec_time_ns = res.exec_time_ns
        kernel.last_profile = res.profile_json
    return out


# revision 31
# speedup vs baseline: 1.1072x; 1.0114x over previous
"""Trainium2 Bass kernel: GQA attention block.

Problem (hardcoded): B=2, T=1024, C=2048, N_HEADS=16, N_KV=4, H=128.
  q = rms_norm(x @ q_kernel); k = rms_norm(x @ k_kernel); v = x @ v_kernel
  q, k: RoPE;  logits = (q/sqrt(H)) @ k^T;  softmax (full, non-causal)
  out = (probs @ v) @ out_kernel

Sharding over 8 cores: core c -> (batch b = c//4, T-slice s = c%4 of 256
query rows).  Each core computes K/V for the full batch (the attention is
non-causal over all 1024 keys) and Q only for its slice; the per-core
[256, 2048] output slices are gathered on host.

On-chip layout is head-major/transposed: xT [C, T], qT/kT [head_dim, t].
The host rolls the key/value sequence so each core's 256 query positions
come first (softmax/AV are permutation-invariant in s; the RoPE tables are
rolled to match), which lets Q-projection read the first 256 columns of the
same resident xT used by K/V.  RMS-norm sums-of-squares become ones-matmul
column sums; RoPE's rotate-half is a constant permutation matmul on the PE
(DVE lanes cannot cross partitions), with sign and 1/sqrt(H) folded into
host-precomputed tables.  Softmax skips max-subtraction (logits are
rms-normed; |logit| < 7).

Scheduling: K-projection accumulates in two T-halves so half-0's PSUM
drain overlaps half-1's matmuls.  Q RoPE is applied per head-pair with
UNSCALED tables as soon as each projection lands (the rms-norm rstd is a
per-token scalar that commutes through the rotation); the rstd is folded
in afterwards by one DVE mul per pair, so nothing but those muls sits on
the q-norm critical path.  qT/kT are bf16 (cheap LDWEIGHTS for the logit
matmuls).  Softmax denominators pre-reduce pairs of s-tiles on DVE/GpSimd
before the ones-matmul.
"""
import os
from contextlib import ExitStack

import numpy as np

import concourse.bacc as bacc
import concourse.bass as bass
import concourse.tile as tile
from concourse import mybir
from concourse.bass_utils import run_bass_kernel_spmd

# problem constants
B, T, C = 2, 1024, 2048
N_HEADS, N_KV, H = 16, 4, 128
G = N_HEADS // N_KV      # 4 q heads per kv head
TL = T // 4              # 256 local q rows per core
P = 128                  # partitions
CT = C // P              # 16 contraction tiles
KM = (N_KV * H) // P     # 4 k m-tiles
ST = T // P              # 8 s-tiles
CB = 4                   # out-proj column blocks of 512
TH = T // 2              # 512: K-proj half width
F32 = mybir.dt.float32
F32R = mybir.dt.float32r
BF16 = mybir.dt.bfloat16
AF = mybir.ActivationFunctionType
EPS = 1e-6
MAX_TIMESCALE = 10000.0


def _r(ap):
    """float32r view (fp22-truncated matmul read) of an fp32 AP."""
    return ap.bitcast(F32R)


def _f(ap):
    """plain-fp32 view of an f32r AP (for DVE/ACT reads)."""
    return ap.bitcast(F32)


def build_nc():
    nc = bacc.Bacc(None, target_bir_lowering=False)
    t_xT = nc.dram_tensor("xT", [P, CT, T], BF16, kind="ExternalInput")
    t_qw = nc.dram_tensor("qw", [CT, P, CT, P], BF16, kind="ExternalInput")
    t_kw = nc.dram_tensor("kw", [P, CT, KM, P], BF16, kind="ExternalInput")
    t_vw = nc.dram_tensor("vw", [P, CT, N_KV * H], BF16, kind="ExternalInput")
    t_ow = nc.dram_tensor("ow", [CB, P, CT, 512], BF16, kind="ExternalInput")
    t_cq = nc.dram_tensor("cq", [P, TL], F32, kind="ExternalInput")
    t_sq = nc.dram_tensor("sq", [P, TL], F32, kind="ExternalInput")
    t_ck = nc.dram_tensor("ck", [P, T], F32, kind="ExternalInput")
    t_sk = nc.dram_tensor("sk", [P, T], F32, kind="ExternalInput")
    t_sw = nc.dram_tensor("sw", [P, P], F32R, kind="ExternalInput")
    t_out = nc.dram_tensor("out", [TL, C], F32, kind="ExternalOutput")

    with tile.TileContext(nc) as tc:
        _emit(tc, t_xT, t_qw, t_kw, t_vw, t_ow,
              t_cq, t_sq, t_ck, t_sk, t_sw, t_out)
    nc.compile()
    return nc


def _emit(tc, t_xT, t_qw, t_kw, t_vw, t_ow, t_cq, t_sq, t_ck, t_sk,
          t_sw, t_out):
    nc = tc.nc

    with ExitStack() as ctx:
        persist = ctx.enter_context(tc.tile_pool(name="persist", bufs=1))
        kT = persist.tile([P, KM, T], BF16)
        ones_b = persist.tile([P, P], BF16)
        nc.vector.memset(ones_b[:], 1.0)
        # manual pools: tiles allocated late so early phases keep headroom
        qTp = tc.alloc_tile_pool(name="qTp", bufs=1, side="right")
        vsbp = tc.alloc_tile_pool(name="vsbp", bufs=1, side="right")
        qT = None
        v_sb = None

        with ExitStack() as xctx:
            xtp = xctx.enter_context(tc.tile_pool(name="xt", bufs=1))
            xT = xtp.tile([P, CT, T], BF16)

            with ExitStack() as tctx:
                tabp = tctx.enter_context(tc.tile_pool(name="tab", bufs=1))
                sw = tabp.tile([P, P], F32R)
                eps_t = tabp.tile([P, 1], F32)
                ckr = tabp.tile([P, T], F32)
                skr = tabp.tile([P, T], F32)
                cq2 = tabp.tile([P, 2, TL], F32)
                sq2 = tabp.tile([P, 2, TL], F32)
                srt = tabp.tile([P, T], F32)
                nc.vector.memset(eps_t[:], EPS)

                with ExitStack() as kctx:
                    wkp = kctx.enter_context(tc.tile_pool(name="wk", bufs=1))
                    ksqp = kctx.enter_context(
                        tc.tile_pool(name="ksq", bufs=2 * KM))
                    tmpp = kctx.enter_context(
                        tc.tile_pool(name="ktmp", bufs=1))
                    vwp = tc.alloc_tile_pool(name="vwp", bufs=1,
                                             side="right")
                    krawp = tc.alloc_tile_pool(name="krawp", bufs=1,
                                               side="right")
                    rtmpp = tc.alloc_tile_pool(name="rtmpp", bufs=4,
                                               side="right")

                    # three DMA rings: kw on sync (first chunk alone so the
                    # first matmul unblocks on 128KB), xT split between the
                    # scalar and gpsimd rings, tables after kw on sync.
                    wkt = wkp.tile([P, CT, KM, P], BF16)
                    nc.sync.dma_start(out=wkt[:, 0, :, :],
                                      in_=t_kw[:, 0, :, :])
                    nc.scalar.dma_start(out=xT[:, 0, 0:TH],
                                        in_=t_xT[:, 0, 0:TH])
                    nc.scalar.dma_start(out=xT[:, 0, TH:T],
                                        in_=t_xT[:, 0, TH:T])
                    nc.gpsimd.dma_start(out=xT[:, 1, :], in_=t_xT[:, 1, :])
                    nc.sync.dma_start(out=wkt[:, 1:4, :, :],
                                      in_=t_kw[:, 1:4, :, :])
                    for c4 in range(1, 4):
                        nc.sync.dma_start(
                            out=wkt[:, 4 * c4:4 * c4 + 4, :, :],
                            in_=t_kw[:, 4 * c4:4 * c4 + 4, :, :])
                    for ct in range(2, CT):
                        eng = nc.scalar if ct % 2 == 0 else nc.gpsimd
                        eng.dma_start(out=xT[:, ct, :], in_=t_xT[:, ct, :])
                    nc.sync.dma_start(out=sw[:], in_=t_sw[:])
                    nc.sync.dma_start(out=ckr[:], in_=t_ck[:])
                    nc.sync.dma_start(out=skr[:], in_=t_sk[:])
                    for h in range(2):
                        nc.sync.dma_start(out=cq2[:, h, :], in_=t_cq[:])
                        nc.sync.dma_start(out=sq2[:, h, :], in_=t_sq[:])

                    kraw = krawp.tile([P, KM, T], F32R)
                    ksqs = [[], []]
                    with ExitStack() as pctx:
                        ppk = pctx.enter_context(
                            tc.tile_pool(name="ppk", bufs=KM, space="PSUM"))
                        # two T-halves: half-0's square/copy drain overlaps
                        # half-1's matmuls
                        pkt = [ppk.tile([P, TH], F32, tag="pk", name="pk")
                               for _ in range(KM)]
                        for h in range(2):
                            lo = h * TH
                            for ct in range(CT):
                                for mt in range(KM):
                                    nc.tensor.matmul(
                                        pkt[mt][:, :],
                                        wkt[:, ct, mt, :],
                                        xT[:, ct, lo:lo + TH],
                                        start=(ct == 0),
                                        stop=(ct == CT - 1))
                            for mt in range(KM):
                                ksq = ksqp.tile([P, TH], BF16, tag="ksq")
                                nc.scalar.square(ksq[:], pkt[mt][:])
                                nc.vector.tensor_copy(
                                    kraw[:, mt, lo:lo + TH], pkt[mt][:])
                                ksqs[h].append(ksq)

                    # ------- Phase Q+V merged (k-rope interleaved) --------
                    with ExitStack() as qctx:
                        wqp = qctx.enter_context(
                            tc.tile_pool(name="wq", bufs=4))
                        qrawp = qctx.enter_context(
                            tc.tile_pool(name="qraw", bufs=1))
                        qtup = qctx.enter_context(
                            tc.tile_pool(name="qtu", bufs=1))
                        qsqp = qctx.enter_context(
                            tc.tile_pool(name="qsq", bufs=2))
                        qtmpp = qctx.enter_context(
                            tc.tile_pool(name="qtmp", bufs=1))
                        qrtmpp = qctx.enter_context(
                            tc.tile_pool(name="qrtmp", bufs=2))

                        qraw = qrawp.tile([P, N_HEADS, TL], F32R)
                        qTu = qtup.tile([P, N_HEADS, TL], F32)
                        # PSUM pools for the swap matmuls and v tiles are
                        # created at mt==2 so their (statically reserved)
                        # banks don't overlap the k-norm column sums.
                        pks = pswk = pswq = ppv = None
                        vt_done = 0

                        def emit_ksum(h):
                            lo = h * TH
                            ks = pks.tile([P, TH], F32, tag="ksum",
                                          name="ksum")
                            for mt in range(KM):
                                nc.tensor.matmul(
                                    ks[:], ones_b[:], ksqs[h][mt][:],
                                    start=(mt == 0), stop=(mt == KM - 1))
                            nc.scalar.activation(srt[:, lo:lo + TH], ks[:],
                                                 AF.Sqrt, bias=eps_t[:],
                                                 scale=1.0 / (N_KV * H))

                        def emit_krstd():
                            rstd = tmpp.tile([P, T], F32, tag="rstd")
                            nc.vector.reciprocal_approx_fast(out=rstd[:],
                                                             in_=srt[:])
                            nc.vector.tensor_mul(ckr[:], ckr[:], rstd[:])
                            nc.vector.tensor_mul(skr[:], skr[:], rstd[:])

                        def emit_ksw(mt, h):
                            lo = h * TH
                            ksw = pswk.tile([P, TH], F32, tag="ksw",
                                            name="ksw")
                            nc.tensor.matmul(
                                ksw[:], _r(sw[:]),
                                _r(kraw[:, mt, lo:lo + TH]),
                                start=True, stop=True)
                            # rope: kT = kraw*ck + swap(kraw)*sk  (both
                            # products in fp32 tmps; the add casts to bf16)
                            t1 = rtmpp.tile([P, TH], F32, tag="rtmp",
                                            name="rtmp")
                            t2 = rtmpp.tile([P, TH], F32, tag="rtmp",
                                            name="rtmp")
                            nc.vector.tensor_mul(t1[:], ksw[:],
                                                 skr[:, lo:lo + TH])
                            nc.vector.tensor_mul(
                                t2[:], _f(kraw[:, mt, lo:lo + TH]),
                                ckr[:, lo:lo + TH])
                            nc.gpsimd.tensor_add(kT[:, mt, lo:lo + TH],
                                                 t2[:], t1[:])

                        def emit_v_mm(tt):
                            pv = ppv.tile([P, N_KV * H], F32, tag="pv",
                                          name="pv")
                            for ct in range(CT):
                                nc.tensor.matmul(
                                    pv[:], xT[:, ct, tt * P:(tt + 1) * P],
                                    vw[:, ct, :],
                                    start=(ct == 0), stop=(ct == CT - 1))
                            return pv

                        def emit_v_tile(tt):
                            pv = emit_v_mm(tt)
                            nc.vector.tensor_copy(v_sb[:, tt, :], pv[:])

                        def emit_qsw_rope(j):
                            # swap matmul + UNSCALED rope for heads 2j,2j+1
                            qsw = pswq.tile([P, 2, TL], F32, tag="qsw",
                                            name="qsw")
                            for hh in range(2):
                                nc.tensor.matmul(
                                    qsw[:, hh, :], _r(sw[:]),
                                    _r(qraw[:, 2 * j + hh, :]),
                                    start=True, stop=True)
                            qtmp = qrtmpp.tile([P, 2, TL], F32,
                                               tag="qrtmp", name="qrtmp")
                            dst = qTu[:, 2 * j:2 * j + 2, :]
                            nc.vector.tensor_mul(qtmp[:], qsw[:], sq2[:])
                            nc.vector.tensor_mul(
                                dst, _f(qraw[:, 2 * j:2 * j + 2, :]),
                                cq2[:])
                            nc.gpsimd.tensor_add(dst, dst, qtmp[:])

                        qsqs = []
                        with ExitStack() as pctx:
                            ppq = pctx.enter_context(
                                tc.tile_pool(name="ppq", bufs=2,
                                             space="PSUM"))
                            pqs = pctx.enter_context(
                                tc.tile_pool(name="pqs", bufs=1,
                                             space="PSUM"))
                            qsum = pqs.tile([P, TL], F32)
                            pks = tc.alloc_tile_pool(name="pks", bufs=2,
                                                     space="PSUM")
                            for mt in range(N_HEADS):
                                wqt = wqp.tile([P, CT, P], BF16, tag="wq")
                                weng = nc.sync if mt % 2 == 0 else nc.gpsimd
                                weng.dma_start(out=wqt[:], in_=t_qw[mt])
                                pq = ppq.tile([P, TL], F32, tag="pq")
                                for ct in range(CT):
                                    nc.tensor.matmul(
                                        pq[:], wqt[:, ct, :],
                                        xT[:, ct, 0:TL],
                                        start=(ct == 0), stop=(ct == CT - 1))
                                qsq = qsqp.tile([P, TL], BF16, tag="qsq")
                                nc.scalar.square(qsq[:], pq[:])
                                nc.vector.tensor_copy(qraw[:, mt, :], pq[:])
                                qsqs.append(qsq)
                                if mt == 0:
                                    emit_ksum(0)
                                elif mt == 1:
                                    emit_ksum(1)
                                    emit_krstd()
                                    pks.release()
                                elif mt == 2:
                                    pswk = tc.alloc_tile_pool(
                                        name="pswk", bufs=2, space="PSUM",
                                        side="right")
                                    pswq = tc.alloc_tile_pool(
                                        name="pswq", bufs=1, space="PSUM",
                                        side="right")
                                    ppv = tc.alloc_tile_pool(
                                        name="ppv", bufs=2, space="PSUM",
                                        side="right")
                                if mt in (2, 4, 6, 8):
                                    kmt = (mt - 2) // 2
                                    emit_ksw(kmt, 0)
                                    emit_ksw(kmt, 1)
                                if mt in (1, 3):
                                    # v weights ride the scalar ring, idle
                                    # once the x chunks are in
                                    if mt == 1:
                                        vw = vwp.tile(
                                            [P, CT, N_KV * H], BF16)
                                    for c4 in (mt - 1, mt):
                                        nc.scalar.dma_start(
                                            out=vw[:, 4 * c4:4 * c4 + 4, :],
                                            in_=t_vw[:, 4 * c4:4 * c4 + 4,
                                                     :])
                                if mt >= 2 and mt % 2 == 0:
                                    emit_qsw_rope((mt - 2) // 2)
                                if mt == 5:
                                    v_sb = vsbp.tile([P, ST, N_KV * H],
                                                     BF16, name="v_sb")
                                if mt >= 5 and mt % 2 == 1:
                                    emit_v_tile(vt_done)
                                    vt_done += 1
                                if mt >= 1:
                                    nc.tensor.matmul(
                                        qsum[:], ones_b[:],
                                        qsqs[mt - 1][:],
                                        start=(mt == 1),
                                        stop=(mt == N_HEADS))
                            nc.tensor.matmul(qsum[:], ones_b[:],
                                             qsqs[N_HEADS - 1][:],
                                             start=False, stop=True)
                            srtq = qtmpp.tile([P, TL], F32, tag="srtq")
                            nc.scalar.activation(srtq[:], qsum[:], AF.Sqrt,
                                                 bias=eps_t[:],
                                                 scale=1.0 / (N_HEADS * H))
                        # tail: last swap+rope, last v tiles as PE filler,
                        # then fold rstd_q into qT (one DVE mul per pair).
                        # v copies are deferred so the DVE reaches the
                        # rstd-fold muls (which gate attention) first.
                        qT = qTp.tile([P, N_HEADS, TL], BF16, name="qT")
                        emit_qsw_rope(7)
                        pv6 = emit_v_mm(6)
                        rstdq2 = qtmpp.tile([P, 2, TL], F32, tag="rstdq2")
                        for h in range(2):
                            nc.vector.reciprocal_approx_fast(
                                out=rstdq2[:, h, :], in_=srtq[:])
                        for j in range(N_HEADS // 2):
                            nc.vector.tensor_mul(
                                qT[:, 2 * j:2 * j + 2, :],
                                qTu[:, 2 * j:2 * j + 2, :], rstdq2[:])
                            if j == 1:
                                nc.vector.tensor_copy(v_sb[:, 6, :],
                                                      pv6[:])
                                pv7 = emit_v_mm(7)
                            elif j == 5:
                                nc.vector.tensor_copy(v_sb[:, 7, :],
                                                      pv7[:])
                        ppv.release()
                        pswq.release()
                        pswk.release()
                        rtmpp.release()
                        krawp.release()
                        vwp.release()

        # ---------------- Phase A: attention ------------------------------
        with ExitStack() as actx:
            attnp = actx.enter_context(tc.tile_pool(name="attn", bufs=1))
            encT = attnp.tile([P, N_HEADS, TL], BF16)
            owp = actx.enter_context(tc.tile_pool(name="ow", bufs=CB))
            # prefetch all out-proj weight blocks during attention
            owts = []
            for cb in range(CB):
                owt = owp.tile([P, CT, 512], BF16, tag="ow", name="ow")
                nc.sync.dma_start(out=owt[:], in_=t_ow[cb])
                owts.append(owt)

            with ExitStack() as kvctx:
                expp = kvctx.enter_context(tc.tile_pool(name="exp", bufs=2))
                exsp = kvctx.enter_context(tc.tile_pool(name="exs", bufs=2))
                rcpp = kvctx.enter_context(tc.tile_pool(name="rcp", bufs=2))
                lp = kvctx.enter_context(
                    tc.tile_pool(name="lp", bufs=3, space="PSUM"))
                sp = kvctx.enter_context(
                    tc.tile_pool(name="sp", bufs=1, space="PSUM"))
                ep = kvctx.enter_context(
                    tc.tile_pool(name="ep", bufs=1, space="PSUM"))

                def emit_se(kh, hlo, ex):
                    # pre-reduce s-tile pairs off the PE, then ones-matmul
                    # the 4 partials into the softmax denominator
                    exs = exsp.tile([P, 4, 2, TL], BF16, tag="exs")
                    for i in range(4):
                        eng = nc.vector if i % 2 == 0 else nc.gpsimd
                        eng.tensor_add(
                            exs[:, i, :, :],
                            ex[:, 2 * i, hlo:hlo + 2, :],
                            ex[:, 2 * i + 1, hlo:hlo + 2, :])
                    S = sp.tile([P, 2, TL], F32, tag="S")
                    for i in range(4):
                        nc.tensor.matmul(
                            S[:], ones_b[:], exs[:, i, :, :],
                            start=(i == 0), stop=(i == 3))
                    # probs @ v  (unnormalized)
                    E = ep.tile([P, 2, TL], F32, tag="E")
                    for st in range(ST):
                        nc.tensor.matmul(
                            E[:], v_sb[:, st, kh * H:(kh + 1) * H],
                            ex[:, st, hlo:hlo + 2, :],
                            start=(st == 0), stop=(st == ST - 1))
                    # normalize while draining PSUM -> SBUF
                    rcp = rcpp.tile([P, 2, TL], F32, tag="rcp")
                    nc.vector.reciprocal_approx_fast(out=rcp[:], in_=S[:])
                    nc.vector.tensor_mul(
                        encT[:, G * kh + hlo:G * kh + hlo + 2, :],
                        E[:], rcp[:])

                # software-pipelined by one pair: pair p's S/E work is
                # emitted after pair p+1's logits, so the PE never waits
                # on ACT exp latency
                pending = None
                for kh in range(N_KV):
                    ex = expp.tile([P, ST, G, TL], BF16, tag="ex")
                    for pair in range(2):
                        hlo = 2 * pair
                        q_rhs = qT[:, G * kh + hlo:G * kh + hlo + 2, :]
                        for st2 in range(ST // 2):
                            L = lp.tile([P, 2, 2, TL], F32, tag="L")
                            for j in range(2):
                                st = st2 * 2 + j
                                nc.tensor.matmul(
                                    L[:, j, :, :],
                                    kT[:, kh, st * P:(st + 1) * P],
                                    q_rhs, start=True, stop=True)
                            nc.scalar.activation(
                                ex[:, st2 * 2:st2 * 2 + 2, hlo:hlo + 2, :],
                                L[:], AF.Exp)
                        if pending is not None:
                            emit_se(*pending)
                        pending = (kh, hlo, ex)
                emit_se(*pending)

            # ---------------- Phase O: output projection ------------------
            with ExitStack() as octx:
                otp = octx.enter_context(tc.tile_pool(name="ot", bufs=6))
                pop = octx.enter_context(
                    tc.tile_pool(name="po", bufs=4, space="PSUM"))
                for cb in range(CB):
                    owt = owts[cb]
                    for tt in range(TL // P):
                        PO = pop.tile([P, 512], F32, tag="PO")
                        for mt in range(CT):
                            nc.tensor.matmul(
                                PO[:], encT[:, mt, tt * P:(tt + 1) * P],
                                owt[:, mt, :],
                                start=(mt == 0), stop=(mt == CT - 1))
                        o = otp.tile([P, 512], F32, tag="o")
                        nc.vector.tensor_copy(o[:], PO[:])
                        eng = nc.sync if (cb * 2 + tt) % 2 == 0 else \
                            nc.scalar
                        eng.dma_start(
                            out=t_out[tt * P:(tt + 1) * P,
                                      cb * 512:(cb + 1) * 512],
                            in_=o[:])
        vsbp.release()
        qTp.release()


# ---------------------------------------------------------------------------
# host side: input prep, sharding, gather
# ---------------------------------------------------------------------------

def _tables():
    fraction = np.arange(0, H, 2, dtype=np.float32) / np.float32(H)
    inv_freq = (1.0 / (MAX_TIMESCALE ** fraction)).astype(np.float32)
    sinusoid = np.arange(T, dtype=np.float32)[:, None] * inv_freq[None, :]
    sinusoid = np.concatenate([sinusoid, sinusoid], axis=-1)  # [T, H]
    sinT = np.sin(sinusoid).T.astype(np.float32)              # [H, T]
    cosT = np.cos(sinusoid).T.astype(np.float32)
    sin_signed = np.concatenate([-sinT[:H // 2], sinT[H // 2:]], axis=0)
    scale = np.float32(1.0) / np.sqrt(np.float32(H)).astype(np.float32)
    return (cosT.copy(), sin_signed.copy(),
            (cosT * scale).astype(np.float32),
            (sin_signed * scale).astype(np.float32))


def make_in_maps(x, q_kernel, k_kernel, v_kernel, out_kernel):
    x = np.ascontiguousarray(np.asarray(x, dtype=np.float32))
    qk = np.asarray(q_kernel, dtype=np.float32)
    kk = np.asarray(k_kernel, dtype=np.float32)
    vk = np.asarray(v_kernel, dtype=np.float32)
    ok = np.asarray(out_kernel, dtype=np.float32)

    import ml_dtypes
    bf16 = ml_dtypes.bfloat16
    qw = np.ascontiguousarray(
        qk.reshape(CT, P, CT, P).transpose(2, 1, 0, 3).astype(bf16))
    kw = np.ascontiguousarray(
        kk.reshape(CT, P, KM, P).transpose(1, 0, 2, 3).astype(bf16))
    vw = np.ascontiguousarray(
        vk.reshape(CT, P, N_KV * H).transpose(1, 0, 2).astype(bf16))
    ow = np.ascontiguousarray(
        ok.reshape(CT, P, CB, 512).transpose(2, 1, 0, 3).astype(
            np.dtype(bf16)))
    ck_h, sk_h, cq_full, sq_full = _tables()
    sw_h = np.zeros((P, P), np.float32)
    sw_h[(np.arange(P) + P // 2) % P, np.arange(P)] = 1.0

    xt = [np.ascontiguousarray(
        x[b].T.reshape(CT, P, T).transpose(1, 0, 2)) for b in range(B)]

    in_maps = []
    for core in range(8):
        b, s = divmod(core, 4)
        t0 = s * TL
        # roll the key/value sequence so this core's query block is first;
        # softmax over s is permutation-invariant, RoPE tables roll along.
        in_maps.append({
            "xT": np.ascontiguousarray(
                np.roll(xt[b], -t0, axis=2).astype(bf16)),
            "qw": qw, "kw": kw, "vw": vw, "ow": ow,
            "cq": np.ascontiguousarray(
                np.roll(cq_full, -t0, axis=1)[:, :TL]),
            "sq": np.ascontiguousarray(
                np.roll(sq_full, -t0, axis=1)[:, :TL]),
            "ck": np.ascontiguousarray(np.roll(ck_h, -t0, axis=1)),
            "sk": np.ascontiguousarray(np.roll(sk_h, -t0, axis=1)),
            "sw": sw_h,
        })
    return in_maps


def _install_trace_shim():
    """Dev-only (KERNEL_TRACE=1): register the NTFF profile hook that this
    agent image's antenv lacks, and skip the artifact cloud upload."""
    import sys
    import types
    try:
        from antenv import axon_hooks  # noqa: F401
        ok = True
    except ImportError:
        try:
            from trn_agent_boot.trn_boot import _ntff_profile_via_ctypes
            hook = _ntff_profile_via_ctypes("/opt/axon/libaxon_pjrt.so")
            m = types.ModuleType("antenv.axon_hooks")
            m.get_axon_ntff_profile_hook = lambda: hook
            m.set_axon_ntff_profile_hook = lambda h: None
            sys.modules["antenv.axon_hooks"] = m
            ok = True
        except Exception as e:  # profiling unavailable; still run
            print(f"trace shim failed: {e!r}")
            ok = False
    if ok:
        import concourse.bass_utils as bu
        bu.upload_artifacts = lambda tmpdir: tmpdir
    return ok


def kernel(x, q_kernel, k_kernel, v_kernel, out_kernel):
    in_maps = make_in_maps(x, q_kernel, k_kernel, v_kernel, out_kernel)
    nc = build_nc()
    trace = bool(os.environ.get("KERNEL_TRACE"))
    kwargs = {}
    if trace:
        trace = _install_trace_shim()
        if trace:
            tdir = os.environ.get("KERNEL_TRACE_DIR")
            if tdir:
                os.makedirs(tdir, exist_ok=True)
                kwargs["tmpdir"] = tdir
    res = run_bass_kernel_spmd(nc, in_maps, core_ids=list(range(8)),
                               trace=trace, **kwargs)
    out = np.zeros((B, T, C), np.float32)
    for core in range(8):
        b, s = divmod(core, 4)
        out[b, s * TL:(s + 1) * TL] = res.results[core]["out"]
    if trace:
        kernel.last_exec_time_ns = res.exec_time_ns
        kernel.last_profile = res.profile_json
    return out
